# revision 11
# baseline (speedup 1.0000x reference)
"""Multi-head attention (batched, key-padding mask) Trainium2 Bass kernel, v2.

Problem: nn_MultiHeadBatched
  q,k,v: [B=4, S=2048, E=1024] fp32; mask: [B, 2048] int32 (key padding)
  16 heads, head_dim 64; torch-Linear style q/k/v/out projections.

Sharding (8 cores): core c handles batch b=c//2 and head group hg=c%2
(8 heads each).  q/k/v projections are column-parallel over the head
group; out-projection is row-parallel - each core produces a partial
[E, Sq] output (bf16) and the host sums the two partials per batch (+bo).

v2 structure (vs v1's head-major slots):
  - Host compacts KV to the valid (mask!=0) positions, pads to a
    multiple of 128 (SKV); pad rows get a -1e30 additive bias folded
    into the ScalarE exp, contributing exactly 0.
  - Head-PAIR strip-units: unit u = (pair k, q-strip s); pair k =
    heads 2k,2k+1 = dh chunk c=k.  Per unit: row-tiled QK for both
    heads (tile_position (0,0)/(64,0) derives from base_partition;
    the two matmuls run concurrently on HW) into one [128,1024] PSUM
    tile [scores_A | scores_B], then nkv EXPs of [128,1024] on ScalarE.
    AV runs with a TWO-unit lag (so exp'd P tiles live ~3 strips and
    the V projection can finish inside units 0-1).
  - Softmax Z comes from an all-ones column appended per head to V
    (row 64 of the [65,512] AV accumulation).  1/Z via DVE
    reciprocal_approx_fast straight from PSUM; GpSimd broadcasts it and
    DVE multiplies (reading PSUM) into the normalized A (bf16, SBUF).
  - Q/K projections for c=1..3 and the V projection are emitted as PE
    filler groups inside the ACT-bound attention units (budgeted so
    every tensor is ready before its first consumer); only c=0's Q/K
    projection runs up front.  Out-projection strips are appended to
    the filler queue as soon as the last pair's strip is normalized,
    overlapping the attention drain.
  - All matmuls bf16 with fp32 PSUM accumulation; no max-subtraction
    (scores/8 ~ N(0,1), exp is safe in fp32).  PSUM budget: scores
    2x[128,1024] (4 banks) + AV accumulators 2x[65,512] (2 banks) +
    projection/out-proj accumulators 2x[128,512] (2 banks) = 8 banks.
"""

import os
import sys

import numpy as np

sys.path.insert(0, "/opt/trn_rl_repo")

import concourse.bass as bass
import concourse.bacc as bacc
import concourse.mybir as mybir
import concourse.tile as tile
from concourse import bass_utils

import ml_dtypes

BF16 = ml_dtypes.bfloat16

B, SQ, E = 4, 2048, 1024
H_TOT, D = 16, 64
HPC = H_TOT // 2            # heads per core (head-group split in 2)
DHC = HPC * D               # 512 projected channels per core
NE = E // 128               # contraction chunks
NC = DHC // 128             # dh chunks per core (= head pairs per core)
NTS = SQ // 512             # 512-wide q strips
NEG = -1.0e30
SCALE = D ** -0.5

N_CORES = 8

_PROGRAM_CACHE = {}
LAST_RESULTS = None


def _kv_chunks(skv):
    out = []
    o = 0
    while o < skv:
        w = min(512, skv - o)
        out.append((o, w))
        o += w
    return out


def build_program(skv):
    """Build + compile the single-core SPMD Bass program for padded KV
    length `skv` (multiple of 128)."""
    if skv in _PROGRAM_CACHE:
        return _PROGRAM_CACHE[skv]

    nkv = skv // 128
    dt = mybir.dt

    nc = bacc.Bacc(
        "TRN2",
        target_bir_lowering=False,
        debug=False,
        enable_asserts=False,
        num_devices=N_CORES,
    )

    # DRAM I/O (per-core shapes)
    qT = nc.dram_tensor("qT", [E, SQ], dt.bfloat16, kind="ExternalInput").ap()
    kT = nc.dram_tensor("kT", [E, skv], dt.bfloat16, kind="ExternalInput").ap()
    vT = nc.dram_tensor("vT", [E, skv], dt.bfloat16, kind="ExternalInput").ap()
    wqT = nc.dram_tensor("wqT", [E, DHC], dt.bfloat16, kind="ExternalInput").ap()
    wkT = nc.dram_tensor("wkT", [E, DHC], dt.bfloat16, kind="ExternalInput").ap()
    wvT = nc.dram_tensor("wvT", [E, DHC], dt.bfloat16, kind="ExternalInput").ap()
    woT = nc.dram_tensor("woT", [DHC, E], dt.bfloat16, kind="ExternalInput").ap()
    mb = nc.dram_tensor("mb", [128, nkv], dt.float32, kind="ExternalInput").ap()
    outT = nc.dram_tensor("outT", [E, SQ], dt.bfloat16, kind="ExternalOutput").ap()
    dbgA = nc.dram_tensor("dbgA", [DHC, SQ], dt.bfloat16, kind="ExternalOutput").ap()

    ts = bass.ts
    kvchunks = _kv_chunks(skv)

    with tile.TileContext(nc) as tc:
        with tc.tile_pool(name="persist", bufs=1) as pp:
            # Persistent SBUF tensors
            wq_sb = [pp.tile([128, DHC], dt.bfloat16, name=f"wq{e}", tag=f"wq{e}") for e in range(NE)]
            wk_sb = [pp.tile([128, DHC], dt.bfloat16, name=f"wk{e}", tag=f"wk{e}") for e in range(NE)]
            wv_sb = [pp.tile([128, DHC], dt.bfloat16, name=f"wv{e}", tag=f"wv{e}") for e in range(NE)]
            wo_sb = [pp.tile([128, E], dt.bfloat16, name=f"wo{c}", tag=f"wo{c}") for c in range(NC)]
            qh_sb = [pp.tile([128, SQ], dt.bfloat16, name=f"qh{c}", tag=f"qh{c}") for c in range(NC)]
            kh_sb = [pp.tile([128, skv], dt.bfloat16, name=f"kh{c}", tag=f"kh{c}") for c in range(NC)]
            # V with per-head interleaved ones column: [kv, 8*(64+1)]
            va_sb = [pp.tile([128, HPC * (D + 1)], dt.bfloat16, name=f"va{j}", tag=f"va{j}") for j in range(nkv)]
            aall_sb = [pp.tile([128, SQ], dt.bfloat16, name=f"aall{c}", tag=f"aall{c}") for c in range(NC)]
            mb_sb = pp.tile([128, nkv], dt.float32, name="mbt", tag="mbt")

            # ones columns of the augmented V (bf16 1.0)
            for j in range(nkv):
                nc.gpsimd.memset(va_sb[j][:, D::D + 1], 1.0)

            # q/k/v raw inputs
            qip = tc.alloc_tile_pool(name="qinp", bufs=1)
            kip = tc.alloc_tile_pool(name="kinp", bufs=1)
            vip = tc.alloc_tile_pool(name="vinp", bufs=1)
            q_sb = [qip.tile([128, SQ], dt.bfloat16, name=f"q{e}", tag=f"q{e}") for e in range(NE)]
            k_sb = [kip.tile([128, skv], dt.bfloat16, name=f"k{e}", tag=f"k{e}") for e in range(NE)]
            v_sb = [vip.tile([128, skv], dt.bfloat16, name=f"v{e}", tag=f"v{e}") for e in range(NE)]

            # DMA order matches first-use order
            for e in range(NE):
                nc.sync.dma_start(wq_sb[e][:], wqT[ts(e, 128), :])
                nc.sync.dma_start(q_sb[e][:], qT[ts(e, 128), :])
            for e in range(NE):
                nc.sync.dma_start(wk_sb[e][:], wkT[ts(e, 128), :])
                nc.sync.dma_start(k_sb[e][:], kT[ts(e, 128), :])
            nc.sync.dma_start(mb_sb[:], mb[:])
            for e in range(NE):
                nc.sync.dma_start(wv_sb[e][:], wvT[ts(e, 128), :])
                nc.sync.dma_start(v_sb[e][:], vT[ts(e, 128), :])
            for c in range(NC):
                nc.sync.dma_start(wo_sb[c][:], woT[ts(c, 128), :])

            # aux PSUM pool: Q/K/V projection accumulators, later out-proj
            aux = tc.alloc_tile_pool(name="aux", bufs=2, space="PSUM")
            opool = tc.alloc_tile_pool(name="opool", bufs=4)

            def qproj_group(c, t):
                """Q projection for dh chunk c, q strip t: qh[c][:, strip]."""
                ps = aux.tile([128, 512], dt.float32, name="aux", tag="aux")
                for e in range(NE):
                    nc.tensor.matmul(
                        ps[:], wq_sb[e][:, ts(c, 128)], q_sb[e][:, ts(t, 512)],
                        start=(e == 0), stop=(e == NE - 1),
                    )
                nc.vector.tensor_copy(qh_sb[c][:, ts(t, 512)], ps[:])

            def kproj_group(c, o, w):
                """K projection for dh chunk c, kv cols [o, o+w)."""
                ps = aux.tile([128, 512], dt.float32, name="aux", tag="aux")
                for e in range(NE):
                    nc.tensor.matmul(
                        ps[:, 0:w], wk_sb[e][:, ts(c, 128)], k_sb[e][:, o:o + w],
                        start=(e == 0), stop=(e == NE - 1),
                    )
                nc.vector.tensor_copy(kh_sb[c][:, o:o + w], ps[:, 0:w])

            def vproj_group(j):
                """V projection for kv chunk j -> va_sb[j] (head-interleaved)."""
                ps = aux.tile([128, 512], dt.float32, name="aux", tag="aux")
                for e in range(NE):
                    nc.tensor.matmul(
                        ps[:], v_sb[e][:, ts(j, 128)], wv_sb[e][:],
                        start=(e == 0), stop=(e == NE - 1),
                    )
                dst = va_sb[j].rearrange("p (h x) -> p h x", x=D + 1)[:, :, 0:D]
                src = ps.rearrange("p (h x) -> p h x", x=D)
                nc.vector.tensor_copy(dst, src)

            def out_group(t, eo):
                """Out-projection for q strip t, output chunk eo."""
                ps = aux.tile([128, 512], dt.float32, name="aux", tag="aux")
                for c in range(NC):
                    nc.tensor.matmul(
                        ps[:], wo_sb[c][:, ts(eo, 128)], aall_sb[c][:, ts(t, 512)],
                        start=(c == 0), stop=(c == NC - 1),
                    )
                ob = opool.tile([128, 512], dt.bfloat16, name="ob", tag="ob")
                nc.vector.tensor_copy(ob[:], ps[:])
                nc.sync.dma_start(outT[ts(eo, 128), ts(t, 512)], ob[:])

            # Filler queue.  Deadlines (enforced by the per-unit budgets
            # below): Qc0 t1 before unit 1; V before unit 2's AV; Q/K c1
            # before unit 4; c2 before unit 8; c3 before unit 12.
            fillers = []
            for t in range(1, NTS):
                fillers.append(lambda t=t: qproj_group(0, t))
            for j in range(nkv):
                fillers.append(lambda j=j: vproj_group(j))
            for c in range(1, NC):
                for t in range(NTS):
                    fillers.append(lambda c=c, t=t: qproj_group(c, t))
                for (o, w) in kvchunks:
                    fillers.append(lambda c=c, o=o, w=w: kproj_group(c, o, w))

            # ---------------- upfront: Qc0 strip 0 + Kc0 ----------------
            qproj_group(0, 0)
            for (o, w) in kvchunks:
                kproj_group(0, o, w)

            # ---------------- attention ----------------
            # 16 QK strip-units + 1 AV drain unit.  AV lags one unit
            # (catch-up of two strips at unit 2, after the V projection
            # fillers of units 0-1 complete).
            units = [(k, s) for k in range(NC) for s in range(NTS)]
            units += [(None, None)]
            budgets = [6, 6, 4, 4] + [2] * 8 + [2, 2, 4, 8] + [99]

            with (
                tc.tile_pool(name="ppool", bufs=1) as ppool,
                tc.tile_pool(name="npool", bufs=2) as npool,
                tc.tile_pool(name="scp", bufs=2, space="PSUM") as scp,
                tc.tile_pool(name="avp", bufs=1, space="PSUM") as avp,
            ):
                p_tiles = {}       # (ui%2, j) -> P tile
                pending = []       # (k, s, ui) strips awaiting AV, FIFO

                def emit_av_full(kp, sp, pui):
                    """AV + normalization for one whole strip (kp, sp)."""
                    a2A = avp.tile([D + 1, 512], dt.float32, name="a2A", tag="a2A")
                    a2B = avp.tile([D + 1, 512], dt.float32, name="a2B", tag="a2B")
                    for j in range(nkv):
                        emit_av_j(a2A, a2B, kp, pui, j)
                    emit_norm(a2A, a2B, kp, sp)

                def emit_av_j(a2A, a2B, kp, pui, j):
                    ptp = p_tiles[(pui % 2, j)]
                    hpA, hpB = 2 * kp, 2 * kp + 1
                    nc.tensor.matmul(
                        a2A[:],
                        va_sb[j][:, hpA * (D + 1):(hpA + 1) * (D + 1)],
                        ptp[:, 0:512],
                        start=(j == 0), stop=(j == nkv - 1),
                    )
                    nc.tensor.matmul(
                        a2B[:],
                        va_sb[j][:, hpB * (D + 1):(hpB + 1) * (D + 1)],
                        ptp[:, 512:1024],
                        start=(j == 0), stop=(j == nkv - 1),
                    )

                def emit_norm(a2A, a2B, kp, sp):
                    # copy A+Z out of PSUM (frees the bank), 1/Z via
                    # fast-approx reciprocal on DVE, broadcast + multiply
                    # on GpSimd (cross-partition write) -> aall bf16
                    for (a2, hp) in ((a2A, 2 * kp), (a2B, 2 * kp + 1)):
                        rp = hp % 2
                        au = npool.tile([D, 512], dt.float32, name="au", tag="au")
                        nc.vector.tensor_copy(au[:], a2[0:D, :])
                        # Z row moved to partition 0: the custom-DVE
                        # reciprocal misreads at base_partition 64
                        z0 = npool.tile([1, 512], dt.float32, name="z0", tag="z0")
                        nc.vector.tensor_copy(z0[:], a2[D:D + 1, :])
                        rz = npool.tile([1, 512], dt.float32, name="rz", tag="rz")
                        nc.vector.reciprocal_approx_fast(rz[:], z0[:])
                        rb = npool.tile([64, 512], dt.float32, name="rb", tag="rb")
                        nc.gpsimd.partition_broadcast(rb[:], rz[:])
                        nc.gpsimd.tensor_mul(
                            aall_sb[kp][rp * 64:(rp + 1) * 64, ts(sp, 512)],
                            au[0:D, :], rb[:],
                        )
                    if kp == NC - 1:
                        # last pair: this q strip is complete -> queue
                        # its out-projection groups
                        for eo in range(NE):
                            fillers.append(lambda t=sp, eo=eo: out_group(t, eo))

                for ui, (k, s) in enumerate(units):
                    budget = budgets[ui] if ui < len(budgets) else 8
                    if ui < 2:
                        avs = []
                    elif ui == 2:
                        avs = [pending.pop(0), pending.pop(0)]
                    else:
                        avs = [pending.pop(0)] if pending else []

                    if k is None:
                        # drain unit: fillers (incl. out-proj) then AVs
                        while fillers and budget > 0:
                            fillers.pop(0)()
                            budget -= 1
                        for (kp, sp, pui) in avs:
                            emit_av_full(kp, sp, pui)
                        continue

                    def emit_qk(j):
                        sc = scp.tile([128, 1024], dt.float32, name="sc", tag="sc")
                        nc.tensor.matmul(
                            sc[:, 0:512],
                            kh_sb[k][0:64, ts(j, 128)],
                            qh_sb[k][0:64, ts(s, 512)],
                            start=True, stop=True,
                        )
                        nc.tensor.matmul(
                            sc[:, 512:1024],
                            kh_sb[k][64:128, ts(j, 128)],
                            qh_sb[k][64:128, ts(s, 512)],
                            start=True, stop=True,
                        )
                        pt = ppool.tile([128, 1024], dt.bfloat16, name="pt",
                                        tag=f"pt{ui % 2}_{j}")
                        p_tiles[(ui % 2, j)] = pt
                        nc.scalar.activation(
                            pt[:], sc[:],
                            mybir.ActivationFunctionType.Exp,
                            bias=mb_sb[:, j:j + 1], scale=SCALE,
                        )

                    # catch-up strip (unit 2 only): AV it in full BEFORE
                    # any of this unit's EXPs reuse the pt{0}_{j} tiles
                    if len(avs) > 1:
                        (kp0, sp0, pui0) = avs.pop(0)
                        emit_av_full(kp0, sp0, pui0)

                    # QK j0, j1 first so ACT has work queued
                    emit_qk(0)
                    if nkv > 1:
                        emit_qk(1)
                        if fillers and budget > 0:
                            fillers.pop(0)()
                            budget -= 1

                    if avs:
                        (kp, sp, pui) = avs[0]
                        a2A = avp.tile([D + 1, 512], dt.float32, name="a2A", tag="a2A")
                        a2B = avp.tile([D + 1, 512], dt.float32, name="a2B", tag="a2B")

                    for j in range(2, nkv):
                        emit_qk(j)
                        if avs:
                            emit_av_j(a2A, a2B, kp, pui, j - 2)
                        if fillers and budget > 0:
                            fillers.pop(0)()
                            budget -= 1
                    if avs:
                        for j in range(max(0, nkv - 2), nkv):
                            emit_av_j(a2A, a2B, kp, pui, j)
                        if nkv == 1:
                            emit_av_j(a2A, a2B, kp, pui, 0)
                        emit_norm(a2A, a2B, kp, sp)

                    # trailing filler pops (covers tiny-nkv edge cases)
                    while fillers and budget > 0:
                        fillers.pop(0)()
                        budget -= 1

                    pending.append((k, s, ui))

                # remaining out-proj groups
                while fillers:
                    fillers.pop(0)()

            for c in range(NC):
                nc.sync.dma_start(dbgA[ts(c, 128), :], aall_sb[c][:])

            opool.release()
            aux.release()
            vip.release()
            kip.release()
            qip.release()

    nc.compile()
    _PROGRAM_CACHE[skv] = nc
    return nc


def make_in_maps(q, k, v, mask, Wq, Wk, Wv, Wo, skv):
    """Host-side shard/compact/transpose/cast. Returns per-core input dicts."""
    in_maps = []
    valid = mask != 0
    for core in range(N_CORES):
        b, hg = core // 2, core % 2
        idx = np.nonzero(valid[b])[0]
        cnt = len(idx)

        kc = np.zeros((skv, E), np.float32)
        vc = np.zeros((skv, E), np.float32)
        kc[:cnt] = k[b][idx]
        vc[:cnt] = v[b][idx]

        mbias = np.zeros((skv,), np.float32)
        mbias[cnt:] = NEG
        # [128, nkv]: column j = kv chunk j
        mb2 = np.ascontiguousarray(mbias.reshape(-1, 128).T)

        rows = slice(hg * DHC, (hg + 1) * DHC)
        in_maps.append(dict(
            qT=np.ascontiguousarray(q[b].T).astype(BF16),
            kT=np.ascontiguousarray(kc.T).astype(BF16),
            vT=np.ascontiguousarray(vc.T).astype(BF16),
            wqT=np.ascontiguousarray(Wq[rows, :].T).astype(BF16),
            wkT=np.ascontiguousarray(Wk[rows, :].T).astype(BF16),
            wvT=np.ascontiguousarray(Wv[rows, :].T).astype(BF16),
            woT=np.ascontiguousarray(Wo[:, rows].T).astype(BF16),
            mb=mb2,
        ))
    return in_maps


def _numpy_fallback(q, k, v, mask, Wq, bq, Wk, bk, Wv, bv, Wo, bo):
    out = np.zeros((B, SQ, E), np.float32)
    for b in range(B):
        qh = (q[b] @ Wq.T + bq).reshape(SQ, H_TOT, D).transpose(1, 0, 2)
        kh = (k[b] @ Wk.T + bk).reshape(-1, H_TOT, D).transpose(1, 0, 2)
        vh = (v[b] @ Wv.T + bv).reshape(-1, H_TOT, D).transpose(1, 0, 2)
        att = np.einsum("hqd,hkd->hqk", qh, kh) * SCALE
        valid = mask[b] != 0
        if not valid.any():
            out[b] = bo
            continue
        att = np.where(valid[None, None, :], att, -np.inf)
        att = att - att.max(-1, keepdims=True)
        att = np.exp(att)
        att /= att.sum(-1, keepdims=True)
        o = np.einsum("hqk,hkd->hqd", att, vh)
        o = o.transpose(1, 0, 2).reshape(SQ, E)
        out[b] = o @ Wo.T + bo
    return out


def kernel(**inputs):
    global LAST_RESULTS
    q = np.asarray(inputs["q"], np.float32)
    k = np.asarray(inputs["k"], np.float32)
    v = np.asarray(inputs["v"], np.float32)
    mask = np.asarray(inputs["mask"])
    Wq, bq = np.asarray(inputs["Wq"], np.float32), np.asarray(inputs["bq"], np.float32)
    Wk, bk = np.asarray(inputs["Wk"], np.float32), np.asarray(inputs["bk"], np.float32)
    Wv, bv = np.asarray(inputs["Wv"], np.float32), np.asarray(inputs["bv"], np.float32)
    Wo, bo = np.asarray(inputs["Wo"], np.float32), np.asarray(inputs["bo"], np.float32)

    if any(np.abs(x).max() > 0 for x in (bq, bk, bv)):
        # q/k/v biases are zero in this problem's setup; a nonzero bias
        # would need the augmented-contraction path, so fall back.
        return _numpy_fallback(q, k, v, mask, Wq, bq, Wk, bk, Wv, bv, Wo, bo)

    valid = mask != 0
    counts = valid.sum(axis=1)
    if counts.max() == 0:
        return np.broadcast_to(bo, (B, SQ, E)).astype(np.float32).copy()

    skv = int(-(-counts.max() // 128) * 128)
    nc = build_program(skv)
    in_maps = make_in_maps(q, k, v, mask, Wq, Wk, Wv, Wo, skv)

    res = bass_utils.run_bass_kernel_spmd(nc, in_maps, core_ids=list(range(N_CORES)))
    LAST_RESULTS = res

    out = np.empty((B, SQ, E), np.float32)
    for b in range(B):
        if counts[b] == 0:
            out[b] = bo
        else:
            p0 = np.asarray(res.results[2 * b]["outT"], np.float32)
            p1 = np.asarray(res.results[2 * b + 1]["outT"], np.float32)
            out[b] = p0.T + p1.T + bo
    return out
